# revision 42
# baseline (speedup 1.0000x reference)
"""Multi-head attention (B=4, T=2048, C=1024, H=16, D=64) on 8 TRN2 cores.

Sharding: core i handles batch b=i//2 and the 8 heads of half hh=i%2.
Each core computes its heads' contribution through the row-sharded output
projection -> partial y [T, C] (fp16); host sums the two partials.

v6: q,k projections in fp8e4 with DoubleRow perf mode (2 c-values per
PE cell -> half the matmul chunks); v projection stays fp16 (fp8 weight
quantization error is systematic across keys and does not average out
of the attention sum, unlike x quantization noise). q/k weights
pre-scaled x32 on host so fp8 normals are used; the scale cancels in
the exp scale (/1024). Rest as v4: x resident in SBUF; pipelined start;
fine-grained emission interleave; multiplicative fp16 causal mask;
reciprocal_approx_fast + K=1 fp16 broadcast matmul for the normalizer;
bias folded into PSUM->SBUF copy.

Per-core layouts (host pre-arranged):
  x8  [128, 4, 2, T] fp8   = x[b].T as [ki, chunk, slot, t], c=256q+128s+ki
  xT  [C, T] fp16          = x[b].T
  wq/wk [128, 4, 4, 2, 128] fp8  [ki, pair, chunk, slot, (head-in-pair, d)]
  wv  [128, 8, 512] fp16   [sbuf-row, c-chunk, (head, d)]
  wpt [512, C] fp16        rows = (local head)*64 + d   (= Wp.T row-slice)
  bp  [C] fp16             bias on even cores, zeros on odd
"""

import os
import sys

import numpy as np

for _p in ("/opt/trn_rl_repo", "/root/.axon_site/_ro/trn_rl_repo"):
    if os.path.isdir(_p) and _p not in sys.path:
        sys.path.append(_p)

import ml_dtypes
import concourse.bass as bass
import concourse.bacc as bacc
import concourse.mybir as mybir
import concourse.tile as tile
from concourse.bass_utils import run_bass_kernel_spmd

B, T, C, H, D = 4, 2048, 1024, 16, 64
HL = H // 2          # heads per core
P = 128
NCH2 = C // 256      # 4 double-row c-chunks
NCH = C // P         # 8 fp16 c-chunks (v path)
NTT = T // 512       # 4 t-tiles of 512
NSB = T // P         # 16 s-blocks of 128
WS = 32.0            # host weight pre-scale (fp8 normal range)
SCALE = (1.0 / 32.0) / (WS * WS)   # 1/sqrt(C) and undo q,k weight scales

F32 = mybir.dt.float32
F16 = mybir.dt.float16
F8 = mybir.dt.float8e4
NP_F8 = ml_dtypes.float8_e4m3
DR = mybir.MatmulPerfMode.DoubleRow


def _build(causal: bool) -> bass.Bass:
    nc = bacc.Bacc("TRN2", target_bir_lowering=False, debug=False, num_devices=8)

    x_d = nc.dram_tensor("x8", [P, NCH2, 2, T], F8, kind="ExternalInput").ap()
    xT_d = nc.dram_tensor("xT", [C, T], F16, kind="ExternalInput").ap()
    # wq+wk packed: one fat 8KB partition line -> few DMA descriptors
    wqk_d = nc.dram_tensor(
        "wqk", [P, 4, 2, NCH2, 2, 2 * D], F8, kind="ExternalInput").ap()
    wv_d = nc.dram_tensor("wv", [P, NCH, HL * D], F16, kind="ExternalInput").ap()
    wpt_d = nc.dram_tensor("wpt", [HL * D, C], F16, kind="ExternalInput").ap()
    bp_d = nc.dram_tensor("bp", [C], F16, kind="ExternalInput").ap()
    y_d = nc.dram_tensor("y", [T, C], F16, kind="ExternalOutput").ap()

    with tile.TileContext(nc) as tc:
        _emit(nc, tc, causal, x_d, xT_d, wqk_d, wv_d, wpt_d, bp_d, y_d)
    nc.compile()
    return nc


def _emit(nc, tc, causal, x_d, xT_d, wqk_d, wv_d, wpt_d, bp_d, y_d):
    from contextlib import ExitStack

    ctx = ExitStack()
    with ctx:
        consts = ctx.enter_context(tc.tile_pool(name="consts", bufs=1))
        x_pool = ctx.enter_context(tc.tile_pool(name="xh", bufs=4))
        x16_pool = ctx.enter_context(tc.tile_pool(name="xh16", bufs=8))
        wv_pool = ctx.enter_context(tc.tile_pool(name="wv", bufs=1))
        q_pool = ctx.enter_context(tc.tile_pool(name="qT", bufs=4))
        k_pool = ctx.enter_context(tc.tile_pool(name="kT", bufs=4))
        v_pool = ctx.enter_context(tc.tile_pool(name="v", bufs=1))
        oc_pool = ctx.enter_context(tc.tile_pool(name="outcat", bufs=4))
        p_pool = ctx.enter_context(tc.tile_pool(name="pT", bufs=6))
        rz_pool = ctx.enter_context(tc.tile_pool(name="rzb", bufs=4))
        zr_pool = ctx.enter_context(tc.tile_pool(name="zr16", bufs=4))
        wpt_pool = ctx.enter_context(tc.tile_pool(name="wpt", bufs=4))
        bpb_pool = ctx.enter_context(tc.tile_pool(name="bpb", bufs=1))
        yt_pool = ctx.enter_context(tc.tile_pool(name="yt", bufs=3))
        psA = ctx.enter_context(tc.tile_pool(name="psA", bufs=2, space="PSUM"))
        psB = ctx.enter_context(tc.tile_pool(name="psB", bufs=4, space="PSUM"))

        # ---- constants ----
        # multiplicative causal mask [128, 2, 128] fp16: 1 where free>=part
        mask01 = None
        if causal:
            mask_f = consts.tile([P, 2, P], F32)
            nc.vector.memset(mask_f, 0.0)
            for _u in range(2):
                nc.gpsimd.affine_select(
                    out=mask_f[:, _u, :], in_=mask_f[:, _u, :],
                    compare_op=mybir.AluOpType.is_ge,
                    fill=-1.0, base=0,
                    pattern=[[1, P]], channel_multiplier=-1,
                )
            # mask_f: 0 on valid, -1 on masked -> mask01 = mask_f + 1
            mask01 = consts.tile([P, 2, P], F16)
            nc.vector.tensor_scalar_add(mask01, mask_f, 1.0)

        ones16 = consts.tile([P, P], F16)
        nc.vector.memset(ones16, 1.0)

        # ---- DMAs: fat contiguous partition lines (4-8KB) minimize the
        # global descriptor count (the DMA path costs ~88ns/descriptor,
        # so descriptors -- not bytes -- set the input feed latency).
        # Order by first use: pair-0 weights + x8 (scores path), then
        # wv + x16 (v path), then the rest. ----
        wqk_t = consts.tile([P, 4, 2, NCH2, 2, 2 * D], F8, name="wqk")
        nc.sync.dma_start(out=wqk_t[:, 0, :, :, :, :], in_=wqk_d[:, 0, :, :, :, :])
        xh = [x_pool.tile([P, 2, T], F8, tag="xh", name=f"xh{c}")
              for c in range(NCH2)]
        for c in range(NCH2):
            nc.sync.dma_start(out=xh[c], in_=x_d[:, c, :, :])
        wv_t = wv_pool.tile([P, NCH, HL * D], F16, tag="wv")
        nc.sync.dma_start(out=wv_t, in_=wv_d)
        xh16 = [x16_pool.tile([P, T], F16, tag="xh16", name=f"xh16_{c}")
                for c in range(NCH)]
        for c in range(NCH):
            nc.sync.dma_start(
                out=xh16[c], in_=xT_d[c * P:(c + 1) * P, :])
        nc.sync.dma_start(
            out=wqk_t[:, 1:4, :, :, :, :], in_=wqk_d[:, 1:4, :, :, :, :])
        wpt_t = [wpt_pool.tile([P, C], F16, tag="wpt", name=f"wpt{i}")
                 for i in range(4)]
        for q in range(4):
            nc.sync.dma_start(out=wpt_t[q], in_=wpt_d[q * P:(q + 1) * P, :])
        bpb16 = bpb_pool.tile([P, C], F16)
        nc.sync.dma_start(
            out=bpb16,
            in_=bass.AP(tensor=bp_d.tensor, offset=0, ap=[[0, P], [1, C]]))
        bpb = bpb_pool.tile([P, C], F32)
        nc.vector.tensor_copy(out=bpb, in_=bpb16)

        # persistent SBUF state
        qT = [None] * 4
        kT = [None] * 4
        # v: [s-part, s-block, head, d + ones]
        v_t = v_pool.tile([P, NSB, HL, D + 1], F16, tag="v")
        nc.vector.memset(v_t[:, :, :, D:], 1.0)
        outcat = [oc_pool.tile([P, T], F16, tag="outcat", name=f"outcat{i}")
                  for i in range(4)]

        # ---- emission helpers: each item emits a small group of PE work,
        # drained between attention s-blocks to keep TensorE dense ----
        _stash = {}

        def v_item(sb, chalf):
            def emit():
                if chalf == 0:
                    vps = psB.tile([P, HL * D], F32, tag="psB", name="vps")
                    _stash[("v", sb)] = vps
                else:
                    vps = _stash.pop(("v", sb))
                for c in range(chalf * 4, chalf * 4 + 4):
                    nc.tensor.matmul(
                        vps, xh16[c][:, sb * P:(sb + 1) * P], wv_t[:, c, :],
                        start=(c == 0), stop=(c == 7),
                        skip_group_check=True)
                if chalf == 1:
                    nc.vector.tensor_copy(
                        out=v_t[:, sb, :, 0:D],
                        in_=vps.rearrange("p (h d) -> p h d", h=HL))
            return emit

        def qk_item(p, tt, which):
            # atomic chain: psum tile lives only within this item, so the
            # transient PSUM ring stays shallow
            def emit():
                tl = psB.tile([P, 512], F32, tag="psB", name=f"{which}ps")
                qki = 0 if which == "q" else 1
                for c in range(NCH2):
                    nc.tensor.matmul(
                        tl, wqk_t[:, p, qki, c, :, :],
                        xh[c][:, :, tt * 512:(tt + 1) * 512],
                        start=(c == 0), stop=(c == NCH2 - 1), perf_mode=DR,
                        skip_group_check=True)
                dst = qT[p] if which == "q" else kT[p]
                nc.vector.tensor_copy(
                    out=dst[:, tt * 512:(tt + 1) * 512], in_=tl)
            return emit

        for _p in range(4):
            qT[_p] = q_pool.tile([P, T], F16, tag="qT", name=f"qT{_p}")
            kT[_p] = k_pool.tile([P, T], F16, tag="kT", name=f"kT{_p}")

        def qk_tt_items(p, tt):
            return [qk_item(p, tt, which) for which in ("q", "k")]

        def proj_item(m):
            def emit():
                yt = yt_pool.tile([P, C], F16, tag="yt", name="yt")
                for n in range(2):
                    yps = psB.tile([P, 512], F32, tag="psB", name="yps")
                    for q in range(4):
                        nc.tensor.matmul(
                            yps,
                            outcat[q][:, m * P:(m + 1) * P],
                            wpt_t[q][:, n * 512:(n + 1) * 512],
                            start=(q == 0), stop=(q == 3),
                            skip_group_check=True)
                    nc.vector.tensor_add(
                        yt[:, n * 512:(n + 1) * 512], yps,
                        bpb[:, n * 512:(n + 1) * 512])
                nc.sync.dma_start(out=y_d[m * P:(m + 1) * P, :], in_=yt)
            return emit

        # ---- preamble: qk(0) t-tile 0, v s-blocks 0-3 ----
        for it in qk_tt_items(0, 0):
            it()
        for sb in range(4):
            for chalf in range(2):
                v_item(sb, chalf)()

        # ---- round-robin schedule: j-tile OUTER, pair INNER. Every
        # (pair, j) unit is ACT-paced (exp dominates the block loop), and
        # fill work (QKV projections, v, output projection) carries
        # global-block deadlines so it spreads across the whole kernel
        # instead of bunching at stage boundaries. ----
        nsb_js = [(4 * (j + 1) if causal else NSB) for j in range(NTT)]
        rstart = [sum(4 * n for n in nsb_js[:j]) for j in range(NTT)]

        ilv = []
        for pp in range(4):
            for j in range(NTT):
                if pp == 0 and j == 0:
                    continue   # preamble
                dl = rstart[j] + pp * nsb_js[j] - 1
                for it in qk_tt_items(pp, j):
                    ilv.append((dl, it))
        for sb in range(4, NSB):
            dl = rstart[sb // 4] + sb - 1
            for chalf in range(2):
                ilv.append((dl, v_item(sb, chalf)))
        ilv.sort(key=lambda e: e[0])

        # weighted spreading: blocks in round j have weight j+1 (late
        # rounds are ACT-heavy and can absorb more PE filler)
        wts = []
        for j in range(NTT):
            wts += [j + 1] * (4 * nsb_js[j])
        wtot = sum(wts)
        wcum = [0.0]
        drained = [0]

        def drain_for_block(b):
            wcum[0] += wts[b] if b < len(wts) else 0
            n0 = len(ilv) + drained[0]
            while ilv and (ilv[0][0] <= b + 1
                           or drained[0] + 1 <= n0 * wcum[0] / wtot):
                ilv.pop(0)[1]()
                drained[0] += 1

        gb = [0]   # global block counter
        deferred = []

        # Z layout per pair: row 32*u, col j (rows 0/32 are valid PE base
        # partitions for the K=1 broadcast matmul); zpair/zscr are
        # transient per-(p,j) scratch
        zr16s = [zr_pool.tile([P, NTT, 512], F16, tag="zr16",
                              name=f"zr16_{i}") for i in range(4)]

        def normalize(j, p):
            bps = psB.tile([P, 512], F32, tag="psB", name="bps")
            for u in range(2):
                k0 = 32 * u
                nc.tensor.matmul(
                    bps[u * D:(u + 1) * D, :],
                    ones16[k0:k0 + 1, 0:D],
                    zr16s[p][k0:k0 + 1, j, :],
                    start=True, stop=True, skip_group_check=True)
            osl = outcat[p][:, j * 512:(j + 1) * 512]
            nc.vector.tensor_mul(osl, osl, bps)

        for j in range(NTT):
            for p in range(4):
                nsb_j = nsb_js[j]
                zr16 = zr16s[p]
                zpair = rz_pool.tile([P, 512], F32, tag="rzb", name="zpair")
                zscr = rz_pool.tile([P, 512], F32, tag="zscr", name="zscr")
                outp = [psB.tile([D + 1, 512], F32, tag="psB",
                                 name=f"outp{u}") for u in range(2)]

                pend = {}

                def emit_pv(i, lo, last, outp=outp, p=p, pend=pend):
                    pts = pend.pop(i)
                    for u in range(2):
                        nc.tensor.matmul(
                            outp[u][:, lo:512],
                            v_t[:, i, p * 2 + u, :],
                            pts[:, u, lo:512],
                            start=(i == 0), stop=last,
                            skip_group_check=True)

                prev = None
                for i in range(nsb_j):
                    drain_for_block(gb[0])
                    gb[0] += 1
                    r = i - 4 * j if causal else -1
                    lo = max(r, 0) * P
                    scs = psA.tile([P, 2, 512], F32, tag="psA", name="scs")
                    pts = p_pool.tile([P, 2, 512], F16, tag="pT", name="pts")
                    pend[i] = pts
                    for u in range(2):
                        dsl = slice(u * D, (u + 1) * D)
                        nc.tensor.matmul(
                            scs[:, u, lo:512],
                            kT[p][dsl, i * P:(i + 1) * P],
                            qT[p][dsl, j * 512 + lo:(j + 1) * 512],
                            start=True, stop=True)
                    nc.scalar.activation(
                        out=pts[:, :, lo:512],
                        in_=scs[:, :, lo:512],
                        func=mybir.ActivationFunctionType.Exp,
                        scale=SCALE)
                    if causal and r >= 0:
                        nc.vector.tensor_mul(
                            pts[:, :, lo:lo + P],
                            pts[:, :, lo:lo + P],
                            mask01)
                    if deferred:
                        if i == 2:
                            deferred.pop(0)()
                            if p == 3 and deferred:
                                deferred.pop(0)()
                        elif p == 3 and i == 5 and deferred:
                            deferred.pop(0)()
                    if prev is not None:
                        emit_pv(*prev)
                    prev = (i, lo, i == nsb_j - 1)
                emit_pv(*prev)

                if p == 3 and j == NTT - 1:
                    # warm-tail epilogue: drain pending fill work first
                    # (its matmuls overlap this epilogue's DVE chain),
                    # then chunk the last tile's normalize+projection in
                    # 256-col halves so the DVE (copies, recip,
                    # normalize) pipelines with the final projection
                    # matmuls and the PE never idles long enough to
                    # lose the HAM clock
                    while ilv:
                        ilv.pop(0)[1]()
                    for hf in range(2):
                        c0, c1 = hf * 256, (hf + 1) * 256
                        for u in range(2):
                            nc.vector.tensor_copy(
                                out=outcat[p][u * D:(u + 1) * D,
                                              j * 512 + c0:j * 512 + c1],
                                in_=outp[u][0:D, c0:c1])
                            nc.vector.tensor_copy(
                                out=zpair[32 * u:32 * u + 1, c0:c1],
                                in_=outp[u][D:D + 1, c0:c1])
                        with nc.allow_low_precision(
                                reason="softmax normalizer"):
                            nc.vector.reciprocal_approx_fast(
                                out=zscr[0:64, c0:c1],
                                in_=zpair[0:64, c0:c1])
                            nc.vector.tensor_copy(
                                out=zr16[0:64, j, c0:c1],
                                in_=zscr[0:64, c0:c1])
                        bps = psB.tile([P, 512], F32, tag="psB",
                                       name="bps")
                        for u in range(2):
                            k0 = 32 * u
                            nc.tensor.matmul(
                                bps[u * D:(u + 1) * D, 0:256],
                                ones16[k0:k0 + 1, 0:D],
                                zr16[k0:k0 + 1, j, c0:c1],
                                start=True, stop=True,
                                skip_group_check=True)
                        osl = outcat[p][:, j * 512 + c0:j * 512 + c1]
                        nc.vector.tensor_mul(osl, osl, bps[:, 0:256])
                        for m in range(4 * j + 2 * hf, 4 * j + 2 * hf + 2):
                            proj_item(m)()
                    continue

                # raw head output + Z row out of PSUM (releases outp ring)
                for u in range(2):
                    nc.vector.tensor_copy(
                        out=outcat[p][u * D:(u + 1) * D,
                                      j * 512:(j + 1) * 512],
                        in_=outp[u][0:D, :])
                    nc.vector.tensor_copy(
                        out=zpair[32 * u:32 * u + 1, :],
                        in_=outp[u][D:D + 1, :])

                # per-(p,j) normalizer chain (keeps DVE smooth and lets
                # the projection of tile j start one round later)
                with nc.allow_low_precision(reason="softmax normalizer"):
                    nc.vector.reciprocal_approx_fast(
                        out=zscr[0:64, :], in_=zpair[0:64, :])
                    nc.vector.tensor_copy(
                        out=zr16[0:64, j, :], in_=zscr[0:64, :])

                if p == 3:
                    def norm3(j=j):
                        normalize(j, 3)
                        for m in range(4 * j, 4 * j + 4):
                            ilv.append((10 ** 9, proj_item(m)))
                    deferred.append(norm3)
                else:
                    deferred.append(lambda j=j, p=p: normalize(j, p))

        for fn in deferred:
            fn()
        while ilv:
            ilv.pop(0)[1]()


_NC_CACHE = {}
LAST_RESULTS = None


def kernel(x, Wq, Wk, Wv, Wp, bp, is_masked, **_unused):
    global LAST_RESULTS
    x = np.asarray(x, np.float32)
    Wq = np.asarray(Wq, np.float32)
    Wk = np.asarray(Wk, np.float32)
    Wv = np.asarray(Wv, np.float32)
    Wp = np.asarray(Wp, np.float32)
    bp = np.asarray(bp, np.float32)
    causal = bool(np.asarray(is_masked).item())

    if causal not in _NC_CACHE:
        _NC_CACHE[causal] = _build(causal)
    nc = _NC_CACHE[causal]

    # host-side layout prep
    def dr8(a2d):
        # [C, M] -> [ki, chunk, slot, M] with c = 256*chunk + 128*slot + ki
        m = a2d.shape[1]
        return np.ascontiguousarray(
            a2d.reshape(NCH2, 2, P, m).transpose(2, 0, 1, 3)).astype(NP_F8)

    wq_r = np.ascontiguousarray(Wq.transpose(1, 0, 2).reshape(C, H * D)) * WS
    wk_r = np.ascontiguousarray(Wk.transpose(1, 0, 2).reshape(C, H * D)) * WS
    wv_r = np.ascontiguousarray(Wv.transpose(1, 0, 2).reshape(C, H * D)).astype(np.float16)
    wpt = np.ascontiguousarray(Wp.T).astype(np.float16)
    bp16 = bp.astype(np.float16)
    zeros = np.zeros_like(bp16)

    xTs = [np.ascontiguousarray(x[b].T).astype(np.float16) for b in range(B)]
    x8s = [dr8(np.ascontiguousarray(x[b].T)) for b in range(B)]
    in_maps = []
    for core in range(8):
        b, hh = core // 2, core % 2
        csl = slice(hh * HL * D, (hh + 1) * HL * D)
        # [ki, pair, qk, chunk, slot, (head-in-pair, d)]
        wqk_c = np.ascontiguousarray(np.stack(
            [np.stack([dr8(w_r[:, csl][:, pp * P:(pp + 1) * P])
                       for w_r in (wq_r, wk_r)], axis=1)
             for pp in range(4)], axis=1))
        # [C, 512] -> [p, chunk, (head, d)]
        wv_c = np.ascontiguousarray(
            wv_r[:, csl].reshape(NCH, P, HL * D).transpose(1, 0, 2))
        in_maps.append({
            "x8": x8s[b],
            "xT": xTs[b],
            "wqk": wqk_c,
            "wv": wv_c,
            "wpt": np.ascontiguousarray(wpt[csl, :]),
            "bp": bp16 if hh == 0 else zeros,
        })

    trace = bool(int(os.environ.get("KERNEL_TRACE", "0")))
    res = run_bass_kernel_spmd(
        nc, in_maps, core_ids=list(range(8)), trace=trace)
    LAST_RESULTS = res

    y = np.empty((B, T, C), np.float32)
    for b in range(B):
        y[b] = (res.results[2 * b]["y"].astype(np.float32)
                + res.results[2 * b + 1]["y"].astype(np.float32))
    return y


# revision 46
# speedup vs baseline: 1.1701x; 1.1701x over previous
"""Multi-head attention (B=4, T=2048, C=1024, H=16, D=64) on 8 TRN2 cores.

Sharding: core i handles batch b=i//2 and the 8 heads of half hh=i%2.
Each core computes its heads' contribution through the row-sharded output
projection -> partial y [T, C] (fp16); host sums the two partials.

v9: q,k projections in fp8e4 with DoubleRow perf mode (2 c-values per
PE cell -> half the matmul chunks); v projection stays fp16 (fp8 weight
quantization error is systematic across keys and does not average out
of the attention sum, unlike x quantization noise). q/k weights
pre-scaled x32 on host so fp8 normals are used; the scale cancels in
the exp scale (/1024). Attention runs j-round-robin (j-tile outer, pair
inner) with globally-deadlined fill work (QKV, v, output projection)
interleaved between attention blocks. The two per-(i,j) score matmuls
run concurrently in disjoint PE row groups (tile_position auto-derived
from the kT base partitions). DMAs use fat contiguous partition lines
(the DMA path is descriptor-count-bound). Warm-tail epilogue chunks the
last tile's normalize+projection so the PE keeps its HAM clock. Causal
mask is a multiplicative fp16 tile on probabilities; softmax normalizer
via ones-column in v + reciprocal_approx_fast + K=1 broadcast matmul;
bias folded into the PSUM->SBUF copy.

Per-core layouts (host pre-arranged):
  x8  [128, 4, 2, T] fp8   = x[b].T as [ki, chunk, slot, t], c=256q+128s+ki
  xT  [C, T] fp16          = x[b].T
  wqk [128, 4, 2, 4, 2, 128] fp8 [ki, pair, q|k, chunk, slot, (head, d)]
  wv  [128, 8, 512] fp16   [sbuf-row, c-chunk, (head, d)]
  wpt [512, C] fp16        rows = (local head)*64 + d   (= Wp.T row-slice)
  bp  [C] fp16             bias on even cores, zeros on odd
"""

import os
import sys

import numpy as np

for _p in ("/opt/trn_rl_repo", "/root/.axon_site/_ro/trn_rl_repo"):
    if os.path.isdir(_p) and _p not in sys.path:
        sys.path.append(_p)

import ml_dtypes
import concourse.bass as bass
import concourse.bacc as bacc
import concourse.mybir as mybir
import concourse.tile as tile
from concourse.bass_utils import run_bass_kernel_spmd

B, T, C, H, D = 4, 2048, 1024, 16, 64
HL = H // 2          # heads per core
P = 128
NCH2 = C // 256      # 4 double-row c-chunks
NCH = C // P         # 8 fp16 c-chunks (v path)
NTT = T // 512       # 4 t-tiles of 512
NSB = T // P         # 16 s-blocks of 128
WS = 32.0            # host weight pre-scale (fp8 normal range)
SCALE = (1.0 / 32.0) / (WS * WS)   # 1/sqrt(C) and undo q,k weight scales

F32 = mybir.dt.float32
F16 = mybir.dt.float16
F8 = mybir.dt.float8e4
NP_F8 = ml_dtypes.float8_e4m3
DR = mybir.MatmulPerfMode.DoubleRow


def _build(causal: bool) -> bass.Bass:
    nc = bacc.Bacc("TRN2", target_bir_lowering=False, debug=False, num_devices=8)

    x_d = nc.dram_tensor("x8", [P, NCH2, 2, T], F8, kind="ExternalInput").ap()
    xT_d = nc.dram_tensor("xT", [C, T], F16, kind="ExternalInput").ap()
    # wq+wk packed: one fat 8KB partition line -> few DMA descriptors
    wqk_d = nc.dram_tensor(
        "wqk", [P, 4, 2, NCH2, 2, 2 * D], F8, kind="ExternalInput").ap()
    wv_d = nc.dram_tensor("wv", [P, NCH, HL * D], F16, kind="ExternalInput").ap()
    wpt_d = nc.dram_tensor("wpt", [HL * D, C], F16, kind="ExternalInput").ap()
    bp_d = nc.dram_tensor("bp", [C], F16, kind="ExternalInput").ap()
    y_d = nc.dram_tensor("y", [T, C], F16, kind="ExternalOutput").ap()

    with tile.TileContext(nc) as tc:
        _emit(nc, tc, causal, x_d, xT_d, wqk_d, wv_d, wpt_d, bp_d, y_d)
    nc.compile()
    return nc


def _emit(nc, tc, causal, x_d, xT_d, wqk_d, wv_d, wpt_d, bp_d, y_d):
    from contextlib import ExitStack

    ctx = ExitStack()
    with ctx:
        consts = ctx.enter_context(tc.tile_pool(name="consts", bufs=1))
        x_pool = ctx.enter_context(tc.tile_pool(name="xh", bufs=4))
        x16_pool = ctx.enter_context(tc.tile_pool(name="xh16", bufs=8))
        wv_pool = ctx.enter_context(tc.tile_pool(name="wv", bufs=1))
        q_pool = ctx.enter_context(tc.tile_pool(name="qT", bufs=4))
        k_pool = ctx.enter_context(tc.tile_pool(name="kT", bufs=4))
        v_pool = ctx.enter_context(tc.tile_pool(name="v", bufs=1))
        oc_pool = ctx.enter_context(tc.tile_pool(name="outcat", bufs=4))
        p_pool = ctx.enter_context(tc.tile_pool(name="pT", bufs=6))
        rz_pool = ctx.enter_context(tc.tile_pool(name="rzb", bufs=4))
        zr_pool = ctx.enter_context(tc.tile_pool(name="zr16", bufs=4))
        wpt_pool = ctx.enter_context(tc.tile_pool(name="wpt", bufs=4))
        bpb_pool = ctx.enter_context(tc.tile_pool(name="bpb", bufs=1))
        yt_pool = ctx.enter_context(tc.tile_pool(name="yt", bufs=3))
        psA = ctx.enter_context(tc.tile_pool(name="psA", bufs=2, space="PSUM"))
        psB = ctx.enter_context(tc.tile_pool(name="psB", bufs=4, space="PSUM"))

        # ---- constants ----
        # multiplicative causal mask [128, 2, 128] fp16: 1 where free>=part
        mask01 = None
        if causal:
            mask_f = consts.tile([P, 2, P], F32)
            nc.vector.memset(mask_f, 0.0)
            for _u in range(2):
                nc.gpsimd.affine_select(
                    out=mask_f[:, _u, :], in_=mask_f[:, _u, :],
                    compare_op=mybir.AluOpType.is_ge,
                    fill=-1.0, base=0,
                    pattern=[[1, P]], channel_multiplier=-1,
                )
            # mask_f: 0 on valid, -1 on masked -> mask01 = mask_f + 1
            mask01 = consts.tile([P, 2, P], F16)
            nc.vector.tensor_scalar_add(mask01, mask_f, 1.0)

        ones16 = consts.tile([P, P], F16)
        nc.vector.memset(ones16, 1.0)

        # ---- DMAs: fat contiguous partition lines (4-8KB) minimize the
        # global descriptor count (the DMA path costs ~88ns/descriptor,
        # so descriptors -- not bytes -- set the input feed latency).
        # Order by first use: pair-0 weights + x8 (scores path), then
        # wv + x16 (v path), then the rest. ----
        wqk_t = consts.tile([P, 4, 2, NCH2, 2, 2 * D], F8, name="wqk")
        nc.sync.dma_start(out=wqk_t[:, 0, :, :, :, :], in_=wqk_d[:, 0, :, :, :, :])
        xh = [x_pool.tile([P, 2, T], F8, tag="xh", name=f"xh{c}")
              for c in range(NCH2)]
        for c in range(NCH2):
            nc.sync.dma_start(out=xh[c], in_=x_d[:, c, :, :])
        wv_t = wv_pool.tile([P, NCH, HL * D], F16, tag="wv")
        nc.sync.dma_start(out=wv_t, in_=wv_d)
        xh16 = [x16_pool.tile([P, T], F16, tag="xh16", name=f"xh16_{c}")
                for c in range(NCH)]
        for c in range(NCH):
            nc.sync.dma_start(
                out=xh16[c], in_=xT_d[c * P:(c + 1) * P, :])
        nc.sync.dma_start(
            out=wqk_t[:, 1:4, :, :, :, :], in_=wqk_d[:, 1:4, :, :, :, :])
        wpt_t = [wpt_pool.tile([P, C], F16, tag="wpt", name=f"wpt{i}")
                 for i in range(4)]
        for q in range(4):
            nc.sync.dma_start(out=wpt_t[q], in_=wpt_d[q * P:(q + 1) * P, :])
        bpb16 = bpb_pool.tile([P, C], F16)
        nc.sync.dma_start(
            out=bpb16,
            in_=bass.AP(tensor=bp_d.tensor, offset=0, ap=[[0, P], [1, C]]))
        bpb = bpb_pool.tile([P, C], F32)
        nc.vector.tensor_copy(out=bpb, in_=bpb16)

        # persistent SBUF state
        qT = [None] * 4
        kT = [None] * 4
        # v: [s-part, s-block, head, d + ones]
        v_t = v_pool.tile([P, NSB, HL, D + 1], F16, tag="v")
        nc.vector.memset(v_t[:, :, :, D:], 1.0)
        outcat = [oc_pool.tile([P, T], F16, tag="outcat", name=f"outcat{i}")
                  for i in range(4)]

        # ---- emission helpers: each item emits a small group of PE work,
        # drained between attention s-blocks to keep TensorE dense ----
        _stash = {}

        def v_item(sb, chalf):
            def emit():
                if chalf == 0:
                    vps = psB.tile([P, HL * D], F32, tag="psB", name="vps")
                    _stash[("v", sb)] = vps
                else:
                    vps = _stash.pop(("v", sb))
                for c in range(chalf * 4, chalf * 4 + 4):
                    nc.tensor.matmul(
                        vps, xh16[c][:, sb * P:(sb + 1) * P], wv_t[:, c, :],
                        start=(c == 0), stop=(c == 7),
                        skip_group_check=True)
                if chalf == 1:
                    nc.vector.tensor_copy(
                        out=v_t[:, sb, :, 0:D],
                        in_=vps.rearrange("p (h d) -> p h d", h=HL))
            return emit

        def qk_item(p, tt, which):
            # atomic chain: psum tile lives only within this item, so the
            # transient PSUM ring stays shallow
            def emit():
                tl = psB.tile([P, 512], F32, tag="psB", name=f"{which}ps")
                qki = 0 if which == "q" else 1
                for c in range(NCH2):
                    nc.tensor.matmul(
                        tl, wqk_t[:, p, qki, c, :, :],
                        xh[c][:, :, tt * 512:(tt + 1) * 512],
                        start=(c == 0), stop=(c == NCH2 - 1), perf_mode=DR,
                        skip_group_check=True)
                dst = qT[p] if which == "q" else kT[p]
                nc.vector.tensor_copy(
                    out=dst[:, tt * 512:(tt + 1) * 512], in_=tl)
            return emit

        for _p in range(4):
            qT[_p] = q_pool.tile([P, T], F16, tag="qT", name=f"qT{_p}")
            kT[_p] = k_pool.tile([P, T], F16, tag="kT", name=f"kT{_p}")

        def qk_tt_items(p, tt):
            return [qk_item(p, tt, which) for which in ("q", "k")]

        def proj_item(m):
            def emit():
                yt = yt_pool.tile([P, C], F16, tag="yt", name="yt")
                for n in range(2):
                    yps = psB.tile([P, 512], F32, tag="psB", name="yps")
                    for q in range(4):
                        nc.tensor.matmul(
                            yps,
                            outcat[q][:, m * P:(m + 1) * P],
                            wpt_t[q][:, n * 512:(n + 1) * 512],
                            start=(q == 0), stop=(q == 3),
                            skip_group_check=True)
                    nc.vector.tensor_add(
                        yt[:, n * 512:(n + 1) * 512], yps,
                        bpb[:, n * 512:(n + 1) * 512])
                nc.sync.dma_start(out=y_d[m * P:(m + 1) * P, :], in_=yt)
            return emit

        # ---- preamble: qk(0) t-tile 0, v s-blocks 0-3 ----
        for it in qk_tt_items(0, 0):
            it()
        for sb in range(4):
            for chalf in range(2):
                v_item(sb, chalf)()

        # ---- round-robin schedule: j-tile OUTER, pair INNER. Every
        # (pair, j) unit is ACT-paced (exp dominates the block loop), and
        # fill work (QKV projections, v, output projection) carries
        # global-block deadlines so it spreads across the whole kernel
        # instead of bunching at stage boundaries. ----
        nsb_js = [(4 * (j + 1) if causal else NSB) for j in range(NTT)]
        rstart = [sum(4 * n for n in nsb_js[:j]) for j in range(NTT)]

        ilv = []
        for pp in range(4):
            for j in range(NTT):
                if pp == 0 and j == 0:
                    continue   # preamble
                dl = rstart[j] + pp * nsb_js[j] - 1
                for it in qk_tt_items(pp, j):
                    ilv.append((dl, it))
        for sb in range(4, NSB):
            dl = rstart[sb // 4] + sb - 1
            for chalf in range(2):
                ilv.append((dl, v_item(sb, chalf)))
        ilv.sort(key=lambda e: e[0])

        # weighted spreading: blocks in round j have weight j+1 (late
        # rounds are ACT-heavy and can absorb more PE filler)
        wts = []
        for j in range(NTT):
            wts += [j + 1] * (4 * nsb_js[j])
        wtot = sum(wts)
        wcum = [0.0]
        drained = [0]

        def drain_for_block(b, frac=1.0):
            wcum[0] += (wts[b] if b < len(wts) else 0) * frac
            n0 = len(ilv) + drained[0]
            while ilv and (ilv[0][0] <= b + 1
                           or drained[0] + 1 <= n0 * wcum[0] / wtot):
                ilv.pop(0)[1]()
                drained[0] += 1

        gb = [0]   # global block counter
        deferred = []

        # Z layout per pair: row 32*u, col j (rows 0/32 are valid PE base
        # partitions for the K=1 broadcast matmul); zpair/zscr are
        # transient per-(p,j) scratch
        zr16s = [zr_pool.tile([P, NTT, 512], F16, tag="zr16",
                              name=f"zr16_{i}") for i in range(4)]

        def normalize(j, p):
            bps = psB.tile([P, 512], F32, tag="psB", name="bps")
            for u in range(2):
                k0 = 32 * u
                nc.tensor.matmul(
                    bps[u * D:(u + 1) * D, :],
                    ones16[k0:k0 + 1, 0:D],
                    zr16s[p][k0:k0 + 1, j, :],
                    start=True, stop=True, skip_group_check=True)
            osl = outcat[p][:, j * 512:(j + 1) * 512]
            nc.vector.tensor_mul(osl, osl, bps)

        for j in range(NTT):
            for p in range(4):
                nsb_j = nsb_js[j]
                zr16 = zr16s[p]
                zpair = rz_pool.tile([P, 512], F32, tag="rzb", name="zpair")
                zscr = rz_pool.tile([P, 512], F32, tag="zscr", name="zscr")
                outp = [psB.tile([D + 1, 512], F32, tag="psB",
                                 name=f"outp{u}") for u in range(2)]

                pend = {}

                def emit_pv(i, lo, last, outp=outp, p=p, pend=pend):
                    pts = pend.pop(i)
                    for u in range(2):
                        nc.tensor.matmul(
                            outp[u][:, lo:512],
                            v_t[:, i, p * 2 + u, :],
                            pts[:, u, lo:512],
                            start=(i == 0), stop=last,
                            skip_group_check=True)

                prev = None
                for i in range(nsb_j):
                    drain_for_block(gb[0], frac=0.5)
                    r = i - 4 * j if causal else -1
                    lo = max(r, 0) * P
                    scs = psA.tile([P, 2, 512], F32, tag="psA", name="scs")
                    pts = p_pool.tile([P, 2, 512], F16, tag="pT", name="pts")
                    pend[i] = pts
                    for u in range(2):
                        dsl = slice(u * D, (u + 1) * D)
                        nc.tensor.matmul(
                            scs[:, u, lo:512],
                            kT[p][dsl, i * P:(i + 1) * P],
                            qT[p][dsl, j * 512 + lo:(j + 1) * 512],
                            start=True, stop=True)
                    nc.scalar.activation(
                        out=pts[:, :, lo:512],
                        in_=scs[:, :, lo:512],
                        func=mybir.ActivationFunctionType.Exp,
                        scale=SCALE)
                    if causal and r >= 0:
                        nc.vector.tensor_mul(
                            pts[:, :, lo:lo + P],
                            pts[:, :, lo:lo + P],
                            mask01)
                    if deferred:
                        if i == 2:
                            deferred.pop(0)()
                            if p == 3 and deferred:
                                deferred.pop(0)()
                        elif p == 3 and i == 5 and deferred:
                            deferred.pop(0)()
                    # second drain point: filler lands between the scores
                    # emission and the PV emission, so the strict-FIFO PE
                    # queue has work while PV's pts dependency resolves
                    drain_for_block(gb[0], frac=0.5)
                    gb[0] += 1
                    if prev is not None:
                        emit_pv(*prev)
                    prev = (i, lo, i == nsb_j - 1)
                emit_pv(*prev)

                if p == 3 and j == NTT - 1:
                    # warm-tail epilogue: drain pending fill work first
                    # (its matmuls overlap this epilogue's DVE chain),
                    # then chunk the last tile's normalize+projection in
                    # 256-col halves so the DVE (copies, recip,
                    # normalize) pipelines with the final projection
                    # matmuls and the PE never idles long enough to
                    # lose the HAM clock
                    while ilv:
                        ilv.pop(0)[1]()
                    for hf in range(2):
                        c0, c1 = hf * 256, (hf + 1) * 256
                        for u in range(2):
                            nc.vector.tensor_copy(
                                out=outcat[p][u * D:(u + 1) * D,
                                              j * 512 + c0:j * 512 + c1],
                                in_=outp[u][0:D, c0:c1])
                            nc.vector.tensor_copy(
                                out=zpair[32 * u:32 * u + 1, c0:c1],
                                in_=outp[u][D:D + 1, c0:c1])
                        with nc.allow_low_precision(
                                reason="softmax normalizer"):
                            nc.vector.reciprocal_approx_fast(
                                out=zscr[0:64, c0:c1],
                                in_=zpair[0:64, c0:c1])
                            nc.vector.tensor_copy(
                                out=zr16[0:64, j, c0:c1],
                                in_=zscr[0:64, c0:c1])
                        bps = psB.tile([P, 512], F32, tag="psB",
                                       name="bps")
                        for u in range(2):
                            k0 = 32 * u
                            nc.tensor.matmul(
                                bps[u * D:(u + 1) * D, 0:256],
                                ones16[k0:k0 + 1, 0:D],
                                zr16[k0:k0 + 1, j, c0:c1],
                                start=True, stop=True,
                                skip_group_check=True)
                        osl = outcat[p][:, j * 512 + c0:j * 512 + c1]
                        nc.vector.tensor_mul(osl, osl, bps[:, 0:256])
                        for m in range(4 * j + 2 * hf, 4 * j + 2 * hf + 2):
                            proj_item(m)()
                    continue

                # raw head output + Z row out of PSUM (releases outp ring)
                for u in range(2):
                    nc.vector.tensor_copy(
                        out=outcat[p][u * D:(u + 1) * D,
                                      j * 512:(j + 1) * 512],
                        in_=outp[u][0:D, :])
                    nc.vector.tensor_copy(
                        out=zpair[32 * u:32 * u + 1, :],
                        in_=outp[u][D:D + 1, :])

                # per-(p,j) normalizer chain (keeps DVE smooth and lets
                # the projection of tile j start one round later)
                with nc.allow_low_precision(reason="softmax normalizer"):
                    nc.vector.reciprocal_approx_fast(
                        out=zscr[0:64, :], in_=zpair[0:64, :])
                    nc.vector.tensor_copy(
                        out=zr16[0:64, j, :], in_=zscr[0:64, :])

                if p == 3:
                    def norm3(j=j):
                        normalize(j, 3)
                        for m in range(4 * j, 4 * j + 4):
                            ilv.append((10 ** 9, proj_item(m)))
                    deferred.append(norm3)
                else:
                    deferred.append(lambda j=j, p=p: normalize(j, p))

        for fn in deferred:
            fn()
        while ilv:
            ilv.pop(0)[1]()


_NC_CACHE = {}
LAST_RESULTS = None


def kernel(x, Wq, Wk, Wv, Wp, bp, is_masked, **_unused):
    global LAST_RESULTS
    x = np.asarray(x, np.float32)
    Wq = np.asarray(Wq, np.float32)
    Wk = np.asarray(Wk, np.float32)
    Wv = np.asarray(Wv, np.float32)
    Wp = np.asarray(Wp, np.float32)
    bp = np.asarray(bp, np.float32)
    causal = bool(np.asarray(is_masked).item())

    if causal not in _NC_CACHE:
        _NC_CACHE[causal] = _build(causal)
    nc = _NC_CACHE[causal]

    # host-side layout prep
    def dr8(a2d):
        # [C, M] -> [ki, chunk, slot, M] with c = 256*chunk + 128*slot + ki
        m = a2d.shape[1]
        return np.ascontiguousarray(
            a2d.reshape(NCH2, 2, P, m).transpose(2, 0, 1, 3)).astype(NP_F8)

    wq_r = np.ascontiguousarray(Wq.transpose(1, 0, 2).reshape(C, H * D)) * WS
    wk_r = np.ascontiguousarray(Wk.transpose(1, 0, 2).reshape(C, H * D)) * WS
    wv_r = np.ascontiguousarray(Wv.transpose(1, 0, 2).reshape(C, H * D)).astype(np.float16)
    wpt = np.ascontiguousarray(Wp.T).astype(np.float16)
    bp16 = bp.astype(np.float16)
    zeros = np.zeros_like(bp16)

    xTs = [np.ascontiguousarray(x[b].T).astype(np.float16) for b in range(B)]
    x8s = [dr8(np.ascontiguousarray(x[b].T)) for b in range(B)]
    in_maps = []
    for core in range(8):
        b, hh = core // 2, core % 2
        csl = slice(hh * HL * D, (hh + 1) * HL * D)
        # [ki, pair, qk, chunk, slot, (head-in-pair, d)]
        wqk_c = np.ascontiguousarray(np.stack(
            [np.stack([dr8(w_r[:, csl][:, pp * P:(pp + 1) * P])
                       for w_r in (wq_r, wk_r)], axis=1)
             for pp in range(4)], axis=1))
        # [C, 512] -> [p, chunk, (head, d)]
        wv_c = np.ascontiguousarray(
            wv_r[:, csl].reshape(NCH, P, HL * D).transpose(1, 0, 2))
        in_maps.append({
            "x8": x8s[b],
            "xT": xTs[b],
            "wqk": wqk_c,
            "wv": wv_c,
            "wpt": np.ascontiguousarray(wpt[csl, :]),
            "bp": bp16 if hh == 0 else zeros,
        })

    trace = bool(int(os.environ.get("KERNEL_TRACE", "0")))
    res = run_bass_kernel_spmd(
        nc, in_maps, core_ids=list(range(8)), trace=trace)
    LAST_RESULTS = res

    y = np.empty((B, T, C), np.float32)
    for b in range(B):
        y[b] = (res.results[2 * b]["y"].astype(np.float32)
                + res.results[2 * b + 1]["y"].astype(np.float32))
    return y


# revision 56
# speedup vs baseline: 1.1908x; 1.0176x over previous
"""Multi-head attention (B=4, T=2048, C=1024, H=16, D=64) on 8 TRN2 cores.

Sharding: core i handles batch b=i//2 and the 8 heads of half hh=i%2.
Each core computes its heads' contribution through the row-sharded output
projection -> partial y [T, C] (fp16); host sums the two partials.

v9: q,k projections in fp8e4 with DoubleRow perf mode (2 c-values per
PE cell -> half the matmul chunks); v projection stays fp16 (fp8 weight
quantization error is systematic across keys and does not average out
of the attention sum, unlike x quantization noise). q/k weights
pre-scaled x32 on host so fp8 normals are used; the scale cancels in
the exp scale (/1024). Attention runs j-round-robin (j-tile outer, pair
inner) with globally-deadlined fill work (QKV, v, output projection)
interleaved between attention blocks. The two per-(i,j) score matmuls
run concurrently in disjoint PE row groups (tile_position auto-derived
from the kT base partitions). DMAs use fat contiguous partition lines
(the DMA path is descriptor-count-bound). Warm-tail epilogue chunks the
last tile's normalize+projection so the PE keeps its HAM clock. Causal
mask is a multiplicative fp16 tile on probabilities; softmax normalizer
via ones-column in v + reciprocal_approx_fast + K=1 broadcast matmul;
bias folded into the PSUM->SBUF copy.

Per-core layouts (host pre-arranged):
  x8  [128, 4, 2, T] fp8   = x[b].T as [ki, chunk, slot, t], c=256q+128s+ki
  xT  [C, T] fp16          = x[b].T
  wqk [128, 4, 2, 4, 2, 128] fp8 [ki, pair, q|k, chunk, slot, (head, d)]
  wv  [128, 8, 512] fp16   [sbuf-row, c-chunk, (head, d)]
  wpt [512, C] fp16        rows = (local head)*64 + d   (= Wp.T row-slice)
  bp  [C] fp16             bias on even cores, zeros on odd
"""

import os
import sys

import numpy as np

for _p in ("/opt/trn_rl_repo", "/root/.axon_site/_ro/trn_rl_repo"):
    if os.path.isdir(_p) and _p not in sys.path:
        sys.path.append(_p)

import ml_dtypes
import concourse.bass as bass
import concourse.bacc as bacc
import concourse.mybir as mybir
import concourse.tile as tile
from concourse.bass_utils import run_bass_kernel_spmd

B, T, C, H, D = 4, 2048, 1024, 16, 64
HL = H // 2          # heads per core
P = 128
NCH2 = C // 256      # 4 double-row c-chunks
NCH = C // P         # 8 fp16 c-chunks (v path)
NTT = T // 512       # 4 t-tiles of 512
NSB = T // P         # 16 s-blocks of 128
WS = 32.0            # host weight pre-scale (fp8 normal range)
SCALE = (1.0 / 32.0) / (WS * WS)   # 1/sqrt(C) and undo q,k weight scales

F32 = mybir.dt.float32
F16 = mybir.dt.float16
F8 = mybir.dt.float8e4
NP_F8 = ml_dtypes.float8_e4m3
DR = mybir.MatmulPerfMode.DoubleRow


def _build(causal: bool) -> bass.Bass:
    nc = bacc.Bacc("TRN2", target_bir_lowering=False, debug=False, num_devices=8)

    x_d = nc.dram_tensor("x8", [P, NCH2, 2, T], F8, kind="ExternalInput").ap()
    xT_d = nc.dram_tensor("xT", [C, T], F16, kind="ExternalInput").ap()
    # wq+wk packed: one fat 8KB partition line -> few DMA descriptors
    wqk_d = nc.dram_tensor(
        "wqk", [P, 4, 2, NCH2, 2, 2 * D], F8, kind="ExternalInput").ap()
    wv_d = nc.dram_tensor("wv", [P, NCH, HL * D], F16, kind="ExternalInput").ap()
    wpt_d = nc.dram_tensor("wpt", [HL * D, C], F16, kind="ExternalInput").ap()
    bp_d = nc.dram_tensor("bp", [C], F16, kind="ExternalInput").ap()
    y_d = nc.dram_tensor("y", [T, C], F16, kind="ExternalOutput").ap()

    with tile.TileContext(nc) as tc:
        _emit(nc, tc, causal, x_d, xT_d, wqk_d, wv_d, wpt_d, bp_d, y_d)
    nc.compile()
    return nc


def _emit(nc, tc, causal, x_d, xT_d, wqk_d, wv_d, wpt_d, bp_d, y_d):
    from contextlib import ExitStack

    ctx = ExitStack()
    with ctx:
        consts = ctx.enter_context(tc.tile_pool(name="consts", bufs=1))
        x_pool = ctx.enter_context(tc.tile_pool(name="xh", bufs=4))
        x16_pool = ctx.enter_context(tc.tile_pool(name="xh16", bufs=8))
        wv_pool = ctx.enter_context(tc.tile_pool(name="wv", bufs=1))
        q_pool = ctx.enter_context(tc.tile_pool(name="qT", bufs=4))
        k_pool = ctx.enter_context(tc.tile_pool(name="kT", bufs=4))
        v_pool = ctx.enter_context(tc.tile_pool(name="v", bufs=1))
        oc_pool = ctx.enter_context(tc.tile_pool(name="outcat", bufs=4))
        p_pool = ctx.enter_context(tc.tile_pool(name="pT", bufs=6))
        rz_pool = ctx.enter_context(tc.tile_pool(name="rzb", bufs=4))
        zr_pool = ctx.enter_context(tc.tile_pool(name="zr16", bufs=4))
        wpt_pool = ctx.enter_context(tc.tile_pool(name="wpt", bufs=4))
        bpb_pool = ctx.enter_context(tc.tile_pool(name="bpb", bufs=1))
        yt_pool = ctx.enter_context(tc.tile_pool(name="yt", bufs=3))
        psA = ctx.enter_context(tc.tile_pool(name="psA", bufs=2, space="PSUM"))
        psB = ctx.enter_context(tc.tile_pool(name="psB", bufs=4, space="PSUM"))

        # ---- constants ----
        # multiplicative causal mask [128, 2, 128] fp16: 1 where free>=part
        mask01 = None
        if causal:
            mask_f = consts.tile([P, 2, P], F32)
            nc.vector.memset(mask_f, 0.0)
            for _u in range(2):
                nc.gpsimd.affine_select(
                    out=mask_f[:, _u, :], in_=mask_f[:, _u, :],
                    compare_op=mybir.AluOpType.is_ge,
                    fill=-1.0, base=0,
                    pattern=[[1, P]], channel_multiplier=-1,
                )
            # mask_f: 0 on valid, -1 on masked -> mask01 = mask_f + 1
            mask01 = consts.tile([P, 2, P], F16)
            nc.vector.tensor_scalar_add(mask01, mask_f, 1.0)

        ones16 = consts.tile([P, P], F16)
        nc.vector.memset(ones16, 1.0)

        # ---- DMAs: fat contiguous partition lines (4-8KB) minimize the
        # global descriptor count (the DMA path costs ~88ns/descriptor,
        # so descriptors -- not bytes -- set the input feed latency).
        # Order by first use: pair-0 weights + x8 (scores path), then
        # wv + x16 (v path), then the rest. ----
        wqk_t = consts.tile([P, 4, 2, NCH2, 2, 2 * D], F8, name="wqk")
        nc.sync.dma_start(out=wqk_t[:, 0, :, :, :, :], in_=wqk_d[:, 0, :, :, :, :])
        xh = [x_pool.tile([P, 2, T], F8, tag="xh", name=f"xh{c}")
              for c in range(NCH2)]
        for c in range(NCH2):
            nc.sync.dma_start(out=xh[c], in_=x_d[:, c, :, :])
        wv_t = wv_pool.tile([P, NCH, HL * D], F16, tag="wv")
        nc.sync.dma_start(out=wv_t, in_=wv_d)
        xh16 = [x16_pool.tile([P, T], F16, tag="xh16", name=f"xh16_{c}")
                for c in range(NCH)]
        for c in range(NCH):
            nc.sync.dma_start(
                out=xh16[c], in_=xT_d[c * P:(c + 1) * P, :])
        nc.sync.dma_start(
            out=wqk_t[:, 1:4, :, :, :, :], in_=wqk_d[:, 1:4, :, :, :, :])
        wpt_t = [wpt_pool.tile([P, C], F16, tag="wpt", name=f"wpt{i}")
                 for i in range(4)]
        for q in range(4):
            nc.sync.dma_start(out=wpt_t[q], in_=wpt_d[q * P:(q + 1) * P, :])
        bpb16 = bpb_pool.tile([P, C], F16)
        nc.sync.dma_start(
            out=bpb16,
            in_=bass.AP(tensor=bp_d.tensor, offset=0, ap=[[0, P], [1, C]]))
        bpb = bpb_pool.tile([P, C], F32)
        nc.vector.tensor_copy(out=bpb, in_=bpb16)

        # ---- HAM warm-up: ~3.5us of dummy matmuls on ones16 during the
        # input-DMA wait, so the PE clock gate is at 8/8 before the first
        # real matmul (otherwise the first ~3.4us of matmuls run at
        # 1.2GHz). Writes go to a psA ring slot that real scores tiles
        # later overwrite with start=True; one DVE read retires the slot
        # for ring reuse. ----
        scs_warm = psA.tile([P, 2, 512], F32, tag="psA", name="scs")
        for _w in range(32):
            nc.tensor.matmul(
                scs_warm[:, 0, 0:P], ones16, ones16,
                start=True, stop=True, skip_group_check=True)
        warm_sink = consts.tile([1, 2], F32)
        nc.vector.tensor_copy(out=warm_sink, in_=scs_warm[0:1, 0, 0:2])

        # persistent SBUF state
        qT = [None] * 4
        kT = [None] * 4
        # v: [s-part, s-block, head, d + ones]
        v_t = v_pool.tile([P, NSB, HL, D + 1], F16, tag="v")
        nc.vector.memset(v_t[:, :, :, D:], 1.0)
        outcat = [oc_pool.tile([P, T], F16, tag="outcat", name=f"outcat{i}")
                  for i in range(4)]

        # ---- emission helpers: each item emits a small group of PE work,
        # drained between attention s-blocks to keep TensorE dense ----
        _stash = {}

        def v_item(sb):
            # atomic: the vps psum buffer is held only within one
            # contiguous emission, so interleaved fillers never find the
            # transient ring occupied by a half-finished v chain
            def emit():
                vps = psB.tile([P, HL * D], F32, tag="psB", name="vps")
                for c in range(NCH):
                    nc.tensor.matmul(
                        vps, xh16[c][:, sb * P:(sb + 1) * P], wv_t[:, c, :],
                        start=(c == 0), stop=(c == NCH - 1),
                        skip_group_check=True)
                nc.vector.tensor_copy(
                    out=v_t[:, sb, :, 0:D],
                    in_=vps.rearrange("p (h d) -> p h d", h=HL))
            return emit

        def qk_item(p, tt, which):
            # atomic chain: psum tile lives only within this item, so the
            # transient PSUM ring stays shallow
            def emit():
                tl = psB.tile([P, 512], F32, tag="psB", name=f"{which}ps")
                qki = 0 if which == "q" else 1
                for c in range(NCH2):
                    nc.tensor.matmul(
                        tl, wqk_t[:, p, qki, c, :, :],
                        xh[c][:, :, tt * 512:(tt + 1) * 512],
                        start=(c == 0), stop=(c == NCH2 - 1), perf_mode=DR,
                        skip_group_check=True)
                dst = qT[p] if which == "q" else kT[p]
                nc.vector.tensor_copy(
                    out=dst[:, tt * 512:(tt + 1) * 512], in_=tl)
            return emit

        for _p in range(4):
            qT[_p] = q_pool.tile([P, T], F16, tag="qT", name=f"qT{_p}")
            kT[_p] = k_pool.tile([P, T], F16, tag="kT", name=f"kT{_p}")

        def qk_tt_items(p, tt):
            return [qk_item(p, tt, which) for which in ("q", "k")]

        def proj_item(m):
            def emit():
                yt = yt_pool.tile([P, C], F16, tag="yt", name="yt")
                for n in range(2):
                    yps = psB.tile([P, 512], F32, tag="psB", name="yps")
                    for q in range(4):
                        nc.tensor.matmul(
                            yps,
                            outcat[q][:, m * P:(m + 1) * P],
                            wpt_t[q][:, n * 512:(n + 1) * 512],
                            start=(q == 0), stop=(q == 3),
                            skip_group_check=True)
                    nc.vector.tensor_add(
                        yt[:, n * 512:(n + 1) * 512], yps,
                        bpb[:, n * 512:(n + 1) * 512])
                nc.sync.dma_start(out=y_d[m * P:(m + 1) * P, :], in_=yt)
            return emit

        # ---- preamble: qk(0) t-tile 0, v s-blocks 0-3 ----
        for it in qk_tt_items(0, 0):
            it()
        for sb in range(4):
            v_item(sb)()

        # ---- round-robin schedule: j-tile OUTER, pair INNER. Every
        # (pair, j) unit is ACT-paced (exp dominates the block loop), and
        # fill work (QKV projections, v, output projection) carries
        # global-block deadlines so it spreads across the whole kernel
        # instead of bunching at stage boundaries. ----
        nsb_js = [(4 * (j + 1) if causal else NSB) for j in range(NTT)]
        rstart = [sum(4 * n for n in nsb_js[:j]) for j in range(NTT)]

        ilv = []
        for pp in range(4):
            for j in range(NTT):
                if pp == 0 and j == 0:
                    continue   # preamble
                dl = rstart[j] + pp * nsb_js[j] - 1
                for it in qk_tt_items(pp, j):
                    ilv.append((dl, it))
        for sb in range(4, NSB):
            dl = rstart[sb // 4] + sb - 1
            ilv.append((dl, v_item(sb)))
        ilv.sort(key=lambda e: e[0])

        # weighted spreading: blocks in round j have weight j+1 (late
        # rounds are ACT-heavy and can absorb more PE filler)
        wts = []
        for j in range(NTT):
            wts += [j + 1] * (4 * nsb_js[j])
        wtot = sum(wts)
        wcum = [0.0]
        drained = [0]

        def drain_for_block(b, frac=1.0):
            wcum[0] += (wts[b] if b < len(wts) else 0) * frac
            n0 = len(ilv) + drained[0]
            while ilv and (ilv[0][0] <= b + 1
                           or drained[0] + 1 <= n0 * wcum[0] / wtot):
                ilv.pop(0)[1]()
                drained[0] += 1

        gb = [0]   # global block counter
        deferred = []

        # Z layout per pair: row 32*u, col j (rows 0/32 are valid PE base
        # partitions for the K=1 broadcast matmul); zpair/zscr are
        # transient per-(p,j) scratch
        zr16s = [zr_pool.tile([P, NTT, 512], F16, tag="zr16",
                              name=f"zr16_{i}") for i in range(4)]

        def normalize(j, p):
            bps = psB.tile([P, 512], F32, tag="psB", name="bps")
            for u in range(2):
                k0 = 32 * u
                nc.tensor.matmul(
                    bps[u * D:(u + 1) * D, :],
                    ones16[k0:k0 + 1, 0:D],
                    zr16s[p][k0:k0 + 1, j, :],
                    start=True, stop=True, skip_group_check=True)
            osl = outcat[p][:, j * 512:(j + 1) * 512]
            nc.vector.tensor_mul(osl, osl, bps)

        for j in range(NTT):
            for p in range(4):
                nsb_j = nsb_js[j]
                zr16 = zr16s[p]
                zpair = rz_pool.tile([P, 512], F32, tag="rzb", name="zpair")
                zscr = rz_pool.tile([P, 512], F32, tag="zscr", name="zscr")
                outp = [psB.tile([D + 1, 512], F32, tag="psB",
                                 name=f"outp{u}") for u in range(2)]

                pend = {}

                def emit_pv(i, lo, last, outp=outp, p=p, pend=pend):
                    pts = pend.pop(i)
                    for u in range(2):
                        nc.tensor.matmul(
                            outp[u][:, lo:512],
                            v_t[:, i, p * 2 + u, :],
                            pts[:, u, lo:512],
                            start=(i == 0), stop=last,
                            skip_group_check=True)

                prev = None
                for i in range(nsb_j):
                    drain_for_block(gb[0])
                    gb[0] += 1
                    r = i - 4 * j if causal else -1
                    lo = max(r, 0) * P
                    scs = psA.tile([P, 2, 512], F32, tag="psA", name="scs")
                    pts = p_pool.tile([P, 2, 512], F16, tag="pT", name="pts")
                    pend[i] = pts
                    for u in range(2):
                        dsl = slice(u * D, (u + 1) * D)
                        nc.tensor.matmul(
                            scs[:, u, lo:512],
                            kT[p][dsl, i * P:(i + 1) * P],
                            qT[p][dsl, j * 512 + lo:(j + 1) * 512],
                            start=True, stop=True)
                    nc.scalar.activation(
                        out=pts[:, :, lo:512],
                        in_=scs[:, :, lo:512],
                        func=mybir.ActivationFunctionType.Exp,
                        scale=SCALE)
                    if causal and r >= 0:
                        nc.vector.tensor_mul(
                            pts[:, :, lo:lo + P],
                            pts[:, :, lo:lo + P],
                            mask01)
                    if deferred:
                        if i == 2:
                            deferred.pop(0)()
                            if p == 3 and deferred:
                                deferred.pop(0)()
                        elif p == 3 and i == 5 and deferred:
                            deferred.pop(0)()
                    if prev is not None:
                        emit_pv(*prev)
                    prev = (i, lo, i == nsb_j - 1)
                emit_pv(*prev)

                if p == 3 and j == NTT - 1:
                    # warm-tail epilogue: drain pending fill work first
                    # (its matmuls overlap this epilogue's DVE chain),
                    # then chunk the last tile's normalize+projection in
                    # 256-col halves so the DVE (copies, recip,
                    # normalize) pipelines with the final projection
                    # matmuls and the PE never idles long enough to
                    # lose the HAM clock
                    while ilv:
                        ilv.pop(0)[1]()
                    for hf in range(2):
                        c0, c1 = hf * 256, (hf + 1) * 256
                        for u in range(2):
                            nc.vector.tensor_copy(
                                out=outcat[p][u * D:(u + 1) * D,
                                              j * 512 + c0:j * 512 + c1],
                                in_=outp[u][0:D, c0:c1])
                            nc.vector.tensor_copy(
                                out=zpair[32 * u:32 * u + 1, c0:c1],
                                in_=outp[u][D:D + 1, c0:c1])
                        with nc.allow_low_precision(
                                reason="softmax normalizer"):
                            nc.vector.reciprocal_approx_fast(
                                out=zscr[0:64, c0:c1],
                                in_=zpair[0:64, c0:c1])
                            nc.vector.tensor_copy(
                                out=zr16[0:64, j, c0:c1],
                                in_=zscr[0:64, c0:c1])
                        bps = psB.tile([P, 512], F32, tag="psB",
                                       name="bps")
                        for u in range(2):
                            k0 = 32 * u
                            nc.tensor.matmul(
                                bps[u * D:(u + 1) * D, 0:256],
                                ones16[k0:k0 + 1, 0:D],
                                zr16[k0:k0 + 1, j, c0:c1],
                                start=True, stop=True,
                                skip_group_check=True)
                        osl = outcat[p][:, j * 512 + c0:j * 512 + c1]
                        nc.vector.tensor_mul(osl, osl, bps[:, 0:256])
                        for m in range(4 * j + 2 * hf, 4 * j + 2 * hf + 2):
                            proj_item(m)()
                    continue

                # raw head output + Z row out of PSUM (releases outp ring)
                for u in range(2):
                    nc.vector.tensor_copy(
                        out=outcat[p][u * D:(u + 1) * D,
                                      j * 512:(j + 1) * 512],
                        in_=outp[u][0:D, :])
                    nc.vector.tensor_copy(
                        out=zpair[32 * u:32 * u + 1, :],
                        in_=outp[u][D:D + 1, :])

                # per-(p,j) normalizer chain (keeps DVE smooth and lets
                # the projection of tile j start one round later)
                with nc.allow_low_precision(reason="softmax normalizer"):
                    nc.vector.reciprocal_approx_fast(
                        out=zscr[0:64, :], in_=zpair[0:64, :])
                    nc.vector.tensor_copy(
                        out=zr16[0:64, j, :], in_=zscr[0:64, :])

                if p == 3:
                    def norm3(j=j):
                        normalize(j, 3)
                        for m in range(4 * j, 4 * j + 4):
                            ilv.append((10 ** 9, proj_item(m)))
                    deferred.append(norm3)
                else:
                    deferred.append(lambda j=j, p=p: normalize(j, p))

        for fn in deferred:
            fn()
        while ilv:
            ilv.pop(0)[1]()


_NC_CACHE = {}
LAST_RESULTS = None


def kernel(x, Wq, Wk, Wv, Wp, bp, is_masked, **_unused):
    global LAST_RESULTS
    x = np.asarray(x, np.float32)
    Wq = np.asarray(Wq, np.float32)
    Wk = np.asarray(Wk, np.float32)
    Wv = np.asarray(Wv, np.float32)
    Wp = np.asarray(Wp, np.float32)
    bp = np.asarray(bp, np.float32)
    causal = bool(np.asarray(is_masked).item())

    if causal not in _NC_CACHE:
        _NC_CACHE[causal] = _build(causal)
    nc = _NC_CACHE[causal]

    # host-side layout prep
    def dr8(a2d):
        # [C, M] -> [ki, chunk, slot, M] with c = 256*chunk + 128*slot + ki
        m = a2d.shape[1]
        return np.ascontiguousarray(
            a2d.reshape(NCH2, 2, P, m).transpose(2, 0, 1, 3)).astype(NP_F8)

    wq_r = np.ascontiguousarray(Wq.transpose(1, 0, 2).reshape(C, H * D)) * WS
    wk_r = np.ascontiguousarray(Wk.transpose(1, 0, 2).reshape(C, H * D)) * WS
    wv_r = np.ascontiguousarray(Wv.transpose(1, 0, 2).reshape(C, H * D)).astype(np.float16)
    wpt = np.ascontiguousarray(Wp.T).astype(np.float16)
    bp16 = bp.astype(np.float16)
    zeros = np.zeros_like(bp16)

    xTs = [np.ascontiguousarray(x[b].T).astype(np.float16) for b in range(B)]
    x8s = [dr8(np.ascontiguousarray(x[b].T)) for b in range(B)]
    in_maps = []
    for core in range(8):
        b, hh = core // 2, core % 2
        csl = slice(hh * HL * D, (hh + 1) * HL * D)
        # [ki, pair, qk, chunk, slot, (head-in-pair, d)]
        wqk_c = np.ascontiguousarray(np.stack(
            [np.stack([dr8(w_r[:, csl][:, pp * P:(pp + 1) * P])
                       for w_r in (wq_r, wk_r)], axis=1)
             for pp in range(4)], axis=1))
        # [C, 512] -> [p, chunk, (head, d)]
        wv_c = np.ascontiguousarray(
            wv_r[:, csl].reshape(NCH, P, HL * D).transpose(1, 0, 2))
        in_maps.append({
            "x8": x8s[b],
            "xT": xTs[b],
            "wqk": wqk_c,
            "wv": wv_c,
            "wpt": np.ascontiguousarray(wpt[csl, :]),
            "bp": bp16 if hh == 0 else zeros,
        })

    trace = bool(int(os.environ.get("KERNEL_TRACE", "0")))
    res = run_bass_kernel_spmd(
        nc, in_maps, core_ids=list(range(8)), trace=trace)
    LAST_RESULTS = res

    y = np.empty((B, T, C), np.float32)
    for b in range(B):
        y[b] = (res.results[2 * b]["y"].astype(np.float32)
                + res.results[2 * b + 1]["y"].astype(np.float32))
    return y
